# revision 7
# baseline (speedup 1.0000x reference)
"""Trainium2 Bass kernel for CompressedLinearFP32.

Computes out = x @ (fp16(int8_w) * fp16(scale))^T + bias, with
x: [4, 2048, 4096] fp32, weight_int8: [11008, 4096] int32 (values in [0,127)),
scale/bias: [11008] fp32. Output [4, 2048, 11008] fp32.

Strategy (tensor-parallel over out_features, 8 cores x 1376):
- Host: dequantize weights exactly like the reference (fp16 product of
  fp16(int) * fp16(scale)), pre-transpose to K-major tiles, cast x to fp16.
- Device (per core): resident W^T shard [4096, 1376] fp16 in SBUF; stream
  x^T token tiles [4096, 128]; 32 K-step matmul accumulation in fp32 PSUM;
  bias added during PSUM->SBUF eviction; write [128, 1376] fp32 tiles out.
"""

import numpy as np

import concourse.bacc as bacc
import concourse.mybir as mybir
import concourse.tile as tile
from concourse import bass_utils

B, S, IN, OUT = 4, 2048, 4096, 11008
NCORES = 8
OUT_SHARD = OUT // NCORES  # 1376
TOKENS = B * S  # 8192
P = 128
KO = IN // P  # 32 k-tiles
TT = TOKENS // P  # 64 token tiles
MM_FREE = 512  # one fp32 PSUM bank

# out-feature chunks per token tile: 512 + 512 + 352
OCHUNKS = []
_o = 0
while _o < OUT_SHARD:
    OCHUNKS.append((_o, min(MM_FREE, OUT_SHARD - _o)))
    _o += MM_FREE

_NC_CACHE = None
LAST_RESULTS = None


def _build_bass():
    nc = bacc.Bacc("TRN2", target_bir_lowering=False, debug=False)
    xt = nc.dram_tensor("xt", (TT, P, KO, P), mybir.dt.float16, kind="ExternalInput")
    wt = nc.dram_tensor("wt", (P, KO, OUT_SHARD), mybir.dt.float16, kind="ExternalInput")
    bias = nc.dram_tensor("bias", (P, OUT_SHARD), mybir.dt.float32, kind="ExternalInput")
    out = nc.dram_tensor("out", (TT, P, OUT_SHARD), mybir.dt.float32, kind="ExternalOutput")

    with tile.TileContext(nc) as tc:
        with (
            tc.tile_pool(name="wpool", bufs=1) as wpool,
            tc.tile_pool(name="bpool", bufs=1) as bpool,
            tc.tile_pool(name="xpool", bufs=3) as xpool,
            tc.tile_pool(name="opool", bufs=3) as opool,
            tc.tile_pool(name="pspool", bufs=6, space="PSUM") as pspool,
        ):
            # PE warm-up: the HAM clock gate holds the PE at 1.2 GHz until
            # ~3.4us of sustained activity. Burn dummy matmuls during the
            # initial DMA wait so real matmuls start at 2.4 GHz.
            warm_l = bpool.tile([P, P], mybir.dt.float16)
            warm_r = bpool.tile([P, MM_FREE], mybir.dt.float16)
            nc.any.memset(warm_l[:], 0.0)
            nc.any.memset(warm_r[:], 0.0)
            warm_ps = pspool.tile([P, MM_FREE], mybir.dt.float32, tag="ps")
            for _ in range(12):
                nc.tensor.matmul(warm_ps[:], warm_l[:], warm_r[:], start=True, stop=True)

            # DMA engine-queue split so streams don't serialize behind each
            # other: w on sync, x tiles on scalar, bias+outputs on gpsimd.
            w_sb = wpool.tile([P, KO, OUT_SHARD], mybir.dt.float16)
            # split the 11MB weight load so early k-tiles land first
            for ko in range(KO):
                nc.sync.dma_start(w_sb[:, ko], wt.ap()[:, ko])
            bias_sb = bpool.tile([P, OUT_SHARD], mybir.dt.float32)
            nc.gpsimd.dma_start(bias_sb[:], bias.ap())

            KO_HEAD = min(4, KO - 1)  # first k-tiles land in their own small DMA
            for tt in range(TT):
                x_sb = xpool.tile([P, KO, P], mybir.dt.float16)
                nc.scalar.dma_start(x_sb[:, :KO_HEAD], xt.ap()[tt][:, :KO_HEAD])
                nc.scalar.dma_start(x_sb[:, KO_HEAD:], xt.ap()[tt][:, KO_HEAD:])
                o_sb = opool.tile([P, OUT_SHARD], mybir.dt.float32)
                # k-outer with the 3 out-chunks' PSUM banks accumulating in
                # parallel: the first matmul only needs w k-tile 0, so the
                # weight-load tail overlaps compute instead of serializing.
                pss = [
                    pspool.tile([P, MM_FREE], mybir.dt.float32, tag="ps", name=f"ps{ci}")
                    for ci in range(len(OCHUNKS))
                ]
                for ko in range(KO):
                    for ci, (o0, osz) in enumerate(OCHUNKS):
                        nc.tensor.matmul(
                            pss[ci][:, :osz],
                            x_sb[:, ko],
                            w_sb[:, ko, o0 : o0 + osz],
                            start=(ko == 0),
                            stop=(ko == KO - 1),
                        )
                for ci, (o0, osz) in enumerate(OCHUNKS):
                    nc.vector.tensor_add(
                        out=o_sb[:, o0 : o0 + osz],
                        in0=pss[ci][:, :osz],
                        in1=bias_sb[:, o0 : o0 + osz],
                    )
                nc.gpsimd.dma_start(out.ap()[tt], o_sb[:])

    nc.compile()
    return nc


def _get_nc():
    global _NC_CACHE
    if _NC_CACHE is None:
        _NC_CACHE = _build_bass()
    return _NC_CACHE


def kernel(x, weight_int8, scale, bias):
    global LAST_RESULTS
    x = np.asarray(x, dtype=np.float32)
    weight_int8 = np.asarray(weight_int8)
    scale = np.asarray(scale, dtype=np.float32)
    bias = np.asarray(bias, dtype=np.float32)

    # x^T tiles: xt[tt, p, ko, t] = x[tt*128+t, ko*128+p]  (fp16)
    x16 = x.reshape(TOKENS, IN).astype(np.float16)
    xt = np.ascontiguousarray(x16.reshape(TT, P, KO, P).transpose(0, 3, 2, 1))

    # dequantized weight, exactly as the reference: fp16(int) * fp16(scale)
    w16 = weight_int8.astype(np.float16) * scale.astype(np.float16)[:, None]

    nc = _get_nc()

    in_maps = []
    for c in range(NCORES):
        wc = w16[c * OUT_SHARD : (c + 1) * OUT_SHARD]  # [1376, 4096]
        # wt[p, ko, o] = wc[o, ko*128+p]
        wtc = np.ascontiguousarray(wc.reshape(OUT_SHARD, KO, P).transpose(2, 1, 0))
        bc = bias[c * OUT_SHARD : (c + 1) * OUT_SHARD]
        bias_rep = np.ascontiguousarray(
            np.broadcast_to(bc[None, :], (P, OUT_SHARD))
        )
        in_maps.append({"xt": xt, "wt": wtc, "bias": bias_rep})

    res = bass_utils.run_bass_kernel_spmd(nc, in_maps, core_ids=list(range(NCORES)))
    LAST_RESULTS = res

    shards = [
        res.results[c]["out"].reshape(TOKENS, OUT_SHARD) for c in range(NCORES)
    ]
    full = np.concatenate(shards, axis=1)
    return np.ascontiguousarray(full.reshape(B, S, OUT), dtype=np.float32)


# revision 9
# speedup vs baseline: 1.0018x; 1.0018x over previous
"""Trainium2 Bass kernel for CompressedLinearFP32.

Computes out = x @ (fp16(int8_w) * fp16(scale))^T + bias, with
x: [4, 2048, 4096] fp32, weight_int8: [11008, 4096] int32 (values in [0,127)),
scale/bias: [11008] fp32. Output [4, 2048, 11008] fp32.

Strategy (tensor-parallel over out_features, 8 cores x 1376):
- Host: dequantize weights exactly like the reference (fp16 product of
  fp16(int) * fp16(scale)), pre-transpose to K-major tiles, cast x to fp16.
- Device (per core): resident W^T shard [4096, 1376] fp16 in SBUF; stream
  x^T token tiles [4096, 128]; 32 K-step matmul accumulation in fp32 PSUM;
  bias added during PSUM->SBUF eviction; write [128, 1376] fp32 tiles out.
"""

import numpy as np

import concourse.bacc as bacc
import concourse.mybir as mybir
import concourse.tile as tile
from concourse import bass_utils

B, S, IN, OUT = 4, 2048, 4096, 11008
NCORES = 8
OUT_SHARD = OUT // NCORES  # 1376
TOKENS = B * S  # 8192
P = 128
KO = IN // P  # 32 k-tiles
TT = TOKENS // P  # 64 token tiles
MM_FREE = 512  # one fp32 PSUM bank

# out-feature chunks per token tile: 512 + 512 + 352
OCHUNKS = []
_o = 0
while _o < OUT_SHARD:
    OCHUNKS.append((_o, min(MM_FREE, OUT_SHARD - _o)))
    _o += MM_FREE

_NC_CACHE = None
LAST_RESULTS = None


def _build_bass():
    nc = bacc.Bacc("TRN2", target_bir_lowering=False, debug=False)
    xt = nc.dram_tensor("xt", (TT, P, KO, P), mybir.dt.float16, kind="ExternalInput")
    wt = nc.dram_tensor("wt", (P, KO, OUT_SHARD), mybir.dt.float16, kind="ExternalInput")
    bias = nc.dram_tensor("bias", (P, OUT_SHARD), mybir.dt.float32, kind="ExternalInput")
    out = nc.dram_tensor("out", (TT, P, OUT_SHARD), mybir.dt.float32, kind="ExternalOutput")

    with tile.TileContext(nc) as tc:
        with (
            tc.tile_pool(name="wpool", bufs=1) as wpool,
            tc.tile_pool(name="bpool", bufs=1) as bpool,
            tc.tile_pool(name="xpool", bufs=3) as xpool,
            tc.tile_pool(name="opool", bufs=3) as opool,
            tc.tile_pool(name="pspool", bufs=7, space="PSUM") as pspool,
        ):
            # PE warm-up: the HAM clock gate holds the PE at 1.2 GHz until
            # ~3.4us of sustained activity. Burn dummy matmuls during the
            # initial DMA wait so real matmuls start at 2.4 GHz.
            warm_l = bpool.tile([P, P], mybir.dt.float16)
            warm_r = bpool.tile([P, MM_FREE], mybir.dt.float16)
            nc.any.memset(warm_l[:], 0.0)
            nc.any.memset(warm_r[:], 0.0)
            warm_ps = pspool.tile([P, MM_FREE], mybir.dt.float32, tag="ps")
            for _ in range(12):
                nc.tensor.matmul(warm_ps[:], warm_l[:], warm_r[:], start=True, stop=True)

            # DMA engine-queue split so streams don't serialize behind each
            # other: w on sync, x tiles on scalar, bias+outputs on gpsimd.
            w_sb = wpool.tile([P, KO, OUT_SHARD], mybir.dt.float16)
            # split the 11MB weight load so early k-tiles land first
            for ko in range(KO):
                nc.sync.dma_start(w_sb[:, ko], wt.ap()[:, ko])
            bias_sb = bpool.tile([P, OUT_SHARD], mybir.dt.float32)
            nc.gpsimd.dma_start(bias_sb[:], bias.ap())

            KO_HEAD = min(4, KO - 1)  # first k-tiles land in their own small DMA
            # k-outer with each out-chunk's PSUM bank accumulating in parallel:
            # the first matmul only needs w k-tile 0, so the weight-load tail
            # overlaps compute instead of serializing. The first TWO token
            # tiles share one k-loop: that halves the weight consumption rate
            # at startup so the HBM weight stream keeps ahead of the PE.
            groups = [[0, 1]] + [[t] for t in range(2, TT)]
            for g in groups:
                xs, osb, pss = [], [], []
                for tt in g:
                    x_sb = xpool.tile([P, KO, P], mybir.dt.float16, tag="x", name=f"x_{tt}")
                    nc.scalar.dma_start(x_sb[:, :KO_HEAD], xt.ap()[tt][:, :KO_HEAD])
                    nc.scalar.dma_start(x_sb[:, KO_HEAD:], xt.ap()[tt][:, KO_HEAD:])
                    xs.append(x_sb)
                    osb.append(
                        opool.tile([P, OUT_SHARD], mybir.dt.float32, tag="o", name=f"o_{tt}")
                    )
                    pss.append(
                        [
                            pspool.tile(
                                [P, MM_FREE], mybir.dt.float32, tag="ps", name=f"ps_{tt}_{ci}"
                            )
                            for ci in range(len(OCHUNKS))
                        ]
                    )
                for ko in range(KO):
                    for gi in range(len(g)):
                        for ci, (o0, osz) in enumerate(OCHUNKS):
                            nc.tensor.matmul(
                                pss[gi][ci][:, :osz],
                                xs[gi][:, ko],
                                w_sb[:, ko, o0 : o0 + osz],
                                start=(ko == 0),
                                stop=(ko == KO - 1),
                            )
                for gi, tt in enumerate(g):
                    for ci, (o0, osz) in enumerate(OCHUNKS):
                        nc.vector.tensor_add(
                            out=osb[gi][:, o0 : o0 + osz],
                            in0=pss[gi][ci][:, :osz],
                            in1=bias_sb[:, o0 : o0 + osz],
                        )
                    nc.gpsimd.dma_start(out.ap()[tt], osb[gi][:])

    nc.compile()
    return nc


def _get_nc():
    global _NC_CACHE
    if _NC_CACHE is None:
        _NC_CACHE = _build_bass()
    return _NC_CACHE


def kernel(x, weight_int8, scale, bias):
    global LAST_RESULTS
    x = np.asarray(x, dtype=np.float32)
    weight_int8 = np.asarray(weight_int8)
    scale = np.asarray(scale, dtype=np.float32)
    bias = np.asarray(bias, dtype=np.float32)

    # x^T tiles: xt[tt, p, ko, t] = x[tt*128+t, ko*128+p]  (fp16)
    x16 = x.reshape(TOKENS, IN).astype(np.float16)
    xt = np.ascontiguousarray(x16.reshape(TT, P, KO, P).transpose(0, 3, 2, 1))

    # dequantized weight, exactly as the reference: fp16(int) * fp16(scale)
    w16 = weight_int8.astype(np.float16) * scale.astype(np.float16)[:, None]

    nc = _get_nc()

    in_maps = []
    for c in range(NCORES):
        wc = w16[c * OUT_SHARD : (c + 1) * OUT_SHARD]  # [1376, 4096]
        # wt[p, ko, o] = wc[o, ko*128+p]
        wtc = np.ascontiguousarray(wc.reshape(OUT_SHARD, KO, P).transpose(2, 1, 0))
        bc = bias[c * OUT_SHARD : (c + 1) * OUT_SHARD]
        bias_rep = np.ascontiguousarray(
            np.broadcast_to(bc[None, :], (P, OUT_SHARD))
        )
        in_maps.append({"xt": xt, "wt": wtc, "bias": bias_rep})

    res = bass_utils.run_bass_kernel_spmd(nc, in_maps, core_ids=list(range(NCORES)))
    LAST_RESULTS = res

    shards = [
        res.results[c]["out"].reshape(TOKENS, OUT_SHARD) for c in range(NCORES)
    ]
    full = np.concatenate(shards, axis=1)
    return np.ascontiguousarray(full.reshape(B, S, OUT), dtype=np.float32)


# revision 11
# speedup vs baseline: 1.0026x; 1.0008x over previous
"""Trainium2 Bass kernel for CompressedLinearFP32.

Computes out = x @ (fp16(int8_w) * fp16(scale))^T + bias, with
x: [4, 2048, 4096] fp32, weight_int8: [11008, 4096] int32 (values in [0,127)),
scale/bias: [11008] fp32. Output [4, 2048, 11008] fp32.

Strategy (tensor-parallel over out_features, 8 cores x 1376):
- Host: dequantize weights exactly like the reference (fp16 product of
  fp16(int) * fp16(scale)), pre-transpose to K-major tiles, cast x to fp16.
- Device (per core): resident W^T shard [4096, 1376] fp16 in SBUF; stream
  x^T token tiles [4096, 128]; 32 K-step matmul accumulation in fp32 PSUM;
  bias added during PSUM->SBUF eviction; write [128, 1376] fp32 tiles out.
"""

import numpy as np

import concourse.bacc as bacc
import concourse.mybir as mybir
import concourse.tile as tile
from concourse import bass_utils

B, S, IN, OUT = 4, 2048, 4096, 11008
NCORES = 8
OUT_SHARD = OUT // NCORES  # 1376
TOKENS = B * S  # 8192
P = 128
KO = IN // P  # 32 k-tiles
TT = TOKENS // P  # 64 token tiles
MM_FREE = 512  # one fp32 PSUM bank

# out-feature chunks per token tile: 512 + 512 + 352
OCHUNKS = []
_o = 0
while _o < OUT_SHARD:
    OCHUNKS.append((_o, min(MM_FREE, OUT_SHARD - _o)))
    _o += MM_FREE

_NC_CACHE = None
LAST_RESULTS = None


def _build_bass():
    nc = bacc.Bacc("TRN2", target_bir_lowering=False, debug=False)
    xt = nc.dram_tensor("xt", (TT, P, KO, P), mybir.dt.float16, kind="ExternalInput")
    wt = nc.dram_tensor("wt", (P, KO, OUT_SHARD), mybir.dt.float16, kind="ExternalInput")
    bias = nc.dram_tensor("bias", (P, OUT_SHARD), mybir.dt.float32, kind="ExternalInput")
    out = nc.dram_tensor("out", (TT, P, OUT_SHARD), mybir.dt.float32, kind="ExternalOutput")

    with tile.TileContext(nc) as tc:
        with (
            tc.tile_pool(name="wpool", bufs=1) as wpool,
            tc.tile_pool(name="bpool", bufs=1) as bpool,
            tc.tile_pool(name="xpool", bufs=3) as xpool,
            tc.tile_pool(name="opool", bufs=3) as opool,
            tc.tile_pool(name="pspool", bufs=7, space="PSUM") as pspool,
        ):
            # DMA engine-queue split so streams don't serialize behind each
            # other: w on sync, x tiles on scalar, bias+outputs on gpsimd.
            w_sb = wpool.tile([P, KO, OUT_SHARD], mybir.dt.float16)
            # split the 11MB weight load so early k-tiles land first
            for ko in range(KO):
                nc.sync.dma_start(w_sb[:, ko], wt.ap()[:, ko])
            bias_sb = bpool.tile([P, OUT_SHARD], mybir.dt.float32)
            nc.gpsimd.dma_start(bias_sb[:], bias.ap())

            KO_HEAD = min(4, KO - 1)  # first k-tiles land in their own small DMA
            # k-outer with each out-chunk's PSUM bank accumulating in parallel:
            # the first matmul only needs w k-tile 0, so the weight-load tail
            # overlaps compute instead of serializing. The first TWO token
            # tiles share one k-loop: that halves the weight consumption rate
            # at startup so the HBM weight stream keeps ahead of the PE.
            blk = bpool.tile([1, 1], mybir.dt.float16)
            groups = [[0, 1]] + [[t] for t in range(2, TT)]
            for gidx, g in enumerate(groups):
                if gidx == 1:
                    # Scalar-engine blocker: later x prefetch DMAs (issued by
                    # the scalar engine, in order) wait here until the whole
                    # weight shard has landed, so the weight stream gets full
                    # HBM bandwidth during the startup race.
                    nc.scalar.copy(out=blk[:], in_=w_sb[:1, KO - 1, :1])
                xs, osb, pss = [], [], []
                for tt in g:
                    x_sb = xpool.tile([P, KO, P], mybir.dt.float16, tag="x", name=f"x_{tt}")
                    nc.scalar.dma_start(x_sb[:, :KO_HEAD], xt.ap()[tt][:, :KO_HEAD])
                    nc.scalar.dma_start(x_sb[:, KO_HEAD:], xt.ap()[tt][:, KO_HEAD:])
                    xs.append(x_sb)
                    osb.append(
                        opool.tile([P, OUT_SHARD], mybir.dt.float32, tag="o", name=f"o_{tt}")
                    )
                    pss.append(
                        [
                            pspool.tile(
                                [P, MM_FREE], mybir.dt.float32, tag="ps", name=f"ps_{tt}_{ci}"
                            )
                            for ci in range(len(OCHUNKS))
                        ]
                    )
                for ko in range(KO):
                    for gi in range(len(g)):
                        for ci, (o0, osz) in enumerate(OCHUNKS):
                            nc.tensor.matmul(
                                pss[gi][ci][:, :osz],
                                xs[gi][:, ko],
                                w_sb[:, ko, o0 : o0 + osz],
                                start=(ko == 0),
                                stop=(ko == KO - 1),
                            )
                for gi, tt in enumerate(g):
                    for ci, (o0, osz) in enumerate(OCHUNKS):
                        nc.vector.tensor_add(
                            out=osb[gi][:, o0 : o0 + osz],
                            in0=pss[gi][ci][:, :osz],
                            in1=bias_sb[:, o0 : o0 + osz],
                        )
                    nc.gpsimd.dma_start(out.ap()[tt], osb[gi][:])

    nc.compile()
    return nc


def _get_nc():
    global _NC_CACHE
    if _NC_CACHE is None:
        _NC_CACHE = _build_bass()
    return _NC_CACHE


def kernel(x, weight_int8, scale, bias):
    global LAST_RESULTS
    x = np.asarray(x, dtype=np.float32)
    weight_int8 = np.asarray(weight_int8)
    scale = np.asarray(scale, dtype=np.float32)
    bias = np.asarray(bias, dtype=np.float32)

    # x^T tiles: xt[tt, p, ko, t] = x[tt*128+t, ko*128+p]  (fp16)
    x16 = x.reshape(TOKENS, IN).astype(np.float16)
    xt = np.ascontiguousarray(x16.reshape(TT, P, KO, P).transpose(0, 3, 2, 1))

    # dequantized weight, exactly as the reference: fp16(int) * fp16(scale)
    w16 = weight_int8.astype(np.float16) * scale.astype(np.float16)[:, None]

    nc = _get_nc()

    in_maps = []
    for c in range(NCORES):
        wc = w16[c * OUT_SHARD : (c + 1) * OUT_SHARD]  # [1376, 4096]
        # wt[p, ko, o] = wc[o, ko*128+p]
        wtc = np.ascontiguousarray(wc.reshape(OUT_SHARD, KO, P).transpose(2, 1, 0))
        bc = bias[c * OUT_SHARD : (c + 1) * OUT_SHARD]
        bias_rep = np.ascontiguousarray(
            np.broadcast_to(bc[None, :], (P, OUT_SHARD))
        )
        in_maps.append({"xt": xt, "wt": wtc, "bias": bias_rep})

    res = bass_utils.run_bass_kernel_spmd(nc, in_maps, core_ids=list(range(NCORES)))
    LAST_RESULTS = res

    shards = [
        res.results[c]["out"].reshape(TOKENS, OUT_SHARD) for c in range(NCORES)
    ]
    full = np.concatenate(shards, axis=1)
    return np.ascontiguousarray(full.reshape(B, S, OUT), dtype=np.float32)
